# revision 6
# baseline (speedup 1.0000x reference)
"""Trainium2 Bass kernel for ChannelCompression:
   y = minmax_norm_spatial(leaky_relu(circulant_1x1_conv(x) + b))

Sharding: pure data parallel over batch (16 batches -> 2 per core x 8 cores).

Per-core strategy (memory-roofline bound: read x once, write y once).
DMA microbenchmarking on this part showed:
  - load-only phase: 4 MiB ops on one HWDGE ring ~333 GB/s (1-2 MiB ~275)
  - mixed load+store phase: 1 MiB ops on separate rings ~367 GB/s total,
    and *larger* load ops degrade the mix (~300 GB/s)
  - store-only phase: ~317 GB/s at any op size
so batch 0's x is loaded with 4 MiB ops (load-only phase) and batch 1's
with 1 MiB ops (overlapped with batch-0 stores on the other ring).

  - View each batch as [C=16, G=8, S=32768] and stack (c,g) onto the 128
    SBUF partitions.  The circulant 16x16 conv becomes one 128x128
    block-structured matmul weight kron(W2.T, I8), so every PE column
    computes all 16 output channels for 8 spatial groups at once.
  - Pass 1 streams x tiles in on the sync (SP HWDGE) ring, matmuls into
    PSUM (fp32), applies leaky-relu (+bias) on ScalarE while copying
    PSUM -> resident *bf16* y chunks (8 MiB/batch), and reduces
    per-partition min/max on DVE (bf16 input = 2x rate).
  - Per-batch stats are folded across the 8 spatial groups via tiny PE
    transposes into free-dim space, reduced, inverted, and broadcast back
    to per-partition scale/bias with two tiny selector matmuls (all
    PSUM->SBUF copies on DVE so the ACT sequencer stays free for stores).
  - Pass 2 normalizes the resident bf16 y on GpSimd into f32 staging
    chunks and streams them out on the scalar (ACT HWDGE) ring, so loads
    and stores live on separate DMA rings and overlap.  Pass 2 of batch 0
    is interleaved with pass 1 of batch 1.
"""

import numpy as np
from contextlib import ExitStack

import concourse.bacc as bacc
import concourse.tile as tile
import concourse.bass as bass
from concourse import mybir
from concourse.bass_utils import run_bass_kernel_spmd

F32 = mybir.dt.float32
BF16 = mybir.dt.bfloat16
AF = mybir.ActivationFunctionType
ALU = mybir.AluOpType
AX = mybir.AxisListType

N_CORES = 8
B, C, H, W = 16, 16, 512, 512
G = 8                   # spatial groups stacked into partitions
BP = B // N_CORES       # batches per core
S_FULL = (H * W) // G   # 32768 spatial elems per group
CH = 2048               # logical column chunk (1 MiB f32 / 0.5 MiB bf16)
N_CH = S_FULL // CH     # chunks per batch (16)
PT = 1024               # columns per PSUM tile (2 banks)
MM = 512                # columns per matmul (1 PSUM bank, fp32 moving max)
XT0 = 8192              # batch-0 load op columns (4 MiB)
EPS = 1e-8
NEG_SLOPE = 0.1


def build_nc():
    nc = bacc.Bacc("TRN2", target_bir_lowering=False)

    xs = nc.dram_tensor("x", [BP, C, G, S_FULL], F32, kind="ExternalInput")
    wbd = nc.dram_tensor("wbd", [128, 128], F32, kind="ExternalInput")
    ident = nc.dram_tensor("ident", [128, 128], F32, kind="ExternalInput")
    sel = nc.dram_tensor("sel", [32, 2, 128], F32, kind="ExternalInput")
    bb = nc.dram_tensor("b128", [128, 1], F32, kind="ExternalInput")
    ys = nc.dram_tensor("y", [BP, C, G, S_FULL], F32, kind="ExternalOutput")

    with tile.TileContext(nc) as tc, ExitStack() as ctx:
        consts = ctx.enter_context(tc.tile_pool(name="consts", bufs=1))
        xpool0 = ctx.enter_context(tc.tile_pool(name="xpool0", bufs=2))
        xpool1 = ctx.enter_context(tc.tile_pool(name="xpool1", bufs=3))
        ypool = ctx.enter_context(tc.tile_pool(name="ypool", bufs=N_CH + 6))
        opool = ctx.enter_context(tc.tile_pool(name="opool", bufs=3))
        spool = ctx.enter_context(tc.tile_pool(name="stats", bufs=4))
        small = ctx.enter_context(tc.tile_pool(name="small", bufs=2))
        psum = ctx.enter_context(tc.tile_pool(name="psum", bufs=3, space="PSUM"))
        psmall = ctx.enter_context(tc.tile_pool(name="psmall", bufs=2, space="PSUM"))

        wbd_sb = consts.tile([128, 128], F32)
        nc.gpsimd.dma_start(out=wbd_sb, in_=wbd[:])
        id_sb = consts.tile([128, 128], F32)
        nc.gpsimd.dma_start(out=id_sb, in_=ident[:])
        sel_sb = consts.tile([32, 2, 128], F32)
        nc.gpsimd.dma_start(out=sel_sb, in_=sel[:])
        b_sb = consts.tile([128, 1], F32)
        nc.gpsimd.dma_start(out=b_sb, in_=bb[:])

        state = {}

        def conv_chunk(bi, j, xt, xoff):
            """Matmul+Prelu chunk j (CH cols) of batch bi from x tile xt
            (chunk starts at column xoff within xt); min/max into stats."""
            st_min, st_max, y_chunks = state[bi]
            yt = ypool.tile([128, CH], BF16, tag="y")
            for p in range(CH // PT):
                pt = psum.tile([128, PT], F32, tag="ps")
                for k in range(PT // MM):
                    c0 = xoff + p * PT + k * MM
                    nc.tensor.matmul(
                        pt[:, k * MM:(k + 1) * MM],
                        wbd_sb,
                        xt[:, c0:c0 + MM],
                        start=True,
                        stop=True,
                    )
                # y = leaky_relu(conv + b): fused PSUM->SBUF(bf16) on ScalarE
                nc.scalar.activation(
                    out=yt[:, p * PT:(p + 1) * PT],
                    in_=pt,
                    func=AF.Prelu,
                    bias=b_sb,
                    scale=1.0,
                    alpha=NEG_SLOPE,
                )
            nc.vector.tensor_reduce(
                out=st_min[:, j:j + 1], in_=yt, axis=AX.X, op=ALU.min
            )
            nc.vector.tensor_reduce(
                out=st_max[:, j:j + 1], in_=yt, axis=AX.X, op=ALU.max
            )
            y_chunks.append(yt)

        def pass1_b0_tile(t):
            """Load a 4 MiB x tile of batch 0 (load-only phase) + compute."""
            xt = xpool0.tile([128, XT0], F32, tag="x0")
            nc.sync.dma_start(out=xt, in_=xs[0, :, :, t * XT0:(t + 1) * XT0])
            for c in range(XT0 // CH):
                conv_chunk(0, t * (XT0 // CH) + c, xt, c * CH)

        def pass1_b1_chunk(j):
            """Load a 1 MiB x chunk of batch 1 (mixed phase) + compute."""
            xt = xpool1.tile([128, CH], F32, tag="x1")
            nc.sync.dma_start(out=xt, in_=xs[1, :, :, j * CH:(j + 1) * CH])
            conv_chunk(1, j, xt, 0)

        def stats_fold(bi):
            """Fold per-partition stats into per-partition scale/bias [128,2].
            All PSUM->SBUF copies on DVE so the ACT sequencer stays free."""
            st_min, st_max = state[bi][:2]
            s2 = small.tile([128, 2], F32, tag="s2")
            nc.vector.tensor_reduce(out=s2[:, 0:1], in_=st_min, axis=AX.X, op=ALU.min)
            nc.vector.tensor_reduce(out=s2[:, 1:2], in_=st_max, axis=AX.X, op=ALU.max)
            # transpose [128,1] stats into free dim (partition 0)
            ptr_min = psmall.tile([1, 128], F32, tag="psm")
            nc.tensor.transpose(ptr_min, s2[:, 0:1], id_sb)
            ptr_max = psmall.tile([1, 128], F32, tag="psm")
            nc.tensor.transpose(ptr_max, s2[:, 1:2], id_sb)
            tl = small.tile([1, 256], F32, tag="tl")
            nc.vector.tensor_copy(tl[:, 0:128], ptr_min)
            nc.vector.tensor_copy(tl[:, 128:256], ptr_max)
            # reduce over the 8 groups (free index p = o*8+g)
            u = small.tile([1, 32], F32, tag="u")
            nc.vector.tensor_reduce(
                out=u[:, 0:16],
                in_=tl[:, 0:128].rearrange("p (o g) -> p o g", g=G),
                axis=AX.X,
                op=ALU.min,
            )
            nc.vector.tensor_reduce(
                out=u[:, 16:32],
                in_=tl[:, 128:256].rearrange("p (o g) -> p o g", g=G),
                axis=AX.X,
                op=ALU.max,
            )
            # scale = 1/(mx-mn+eps); nbias = -mn*scale
            vv = small.tile([1, 16], F32, tag="vv")
            nc.vector.scalar_tensor_tensor(
                out=vv, in0=u[:, 16:32], scalar=EPS, in1=u[:, 0:16],
                op0=ALU.add, op1=ALU.subtract,
            )
            pk = small.tile([1, 32], F32, tag="pk")
            nc.vector.reciprocal(out=pk[:, 0:16], in_=vv)
            nc.vector.scalar_tensor_tensor(
                out=pk[:, 16:32], in0=u[:, 0:16], scalar=-1.0, in1=pk[:, 0:16],
                op0=ALU.mult, op1=ALU.mult,
            )
            # broadcast [1,32] free-dim -> per-partition [128,2] via transpose
            # + selector matmuls (sel[k,0,p]=d(k==p//8), sel[k,1,p]=d(k-16==p//8))
            pz = psmall.tile([32, 1], F32, tag="psm")
            nc.tensor.transpose(pz, pk, id_sb[0:1, 0:1])
            zs = small.tile([32, 1], F32, tag="zs")
            nc.vector.tensor_copy(zs, pz)
            pb1 = psmall.tile([128, 1], F32, tag="psm")
            nc.tensor.matmul(pb1, sel_sb[:, 0, :], zs, start=True, stop=True)
            pb2 = psmall.tile([128, 1], F32, tag="psm")
            nc.tensor.matmul(pb2, sel_sb[:, 1, :], zs, start=True, stop=True)
            sc = small.tile([128, 2], F32, tag="sc")
            nc.vector.tensor_copy(sc[:, 0:1], pb1)
            nc.vector.tensor_copy(sc[:, 1:2], pb2)
            return sc

        def pass2_chunk(bi, j, sc):
            """Normalize resident bf16 y chunk (GpSimd) into f32 staging and
            stream out on the scalar (ACT HWDGE) ring."""
            y_chunks = state[bi][2]
            ot = opool.tile([128, CH], F32, tag="o")
            nc.gpsimd.tensor_scalar(
                out=ot,
                in0=y_chunks[j],
                scalar1=sc[:, 0:1],
                scalar2=sc[:, 1:2],
                op0=ALU.mult,
                op1=ALU.add,
            )
            nc.scalar.dma_start(out=ys[bi, :, :, j * CH:(j + 1) * CH], in_=ot)

        for bi in range(BP):
            state[bi] = (
                spool.tile([128, N_CH], F32, tag="stmin", name=f"stmin{bi}"),
                spool.tile([128, N_CH], F32, tag="stmax", name=f"stmax{bi}"),
                [],
            )
        # software pipeline: pass1(b0) with 4 MiB loads; fold(b0); then
        # interleave pass2(b0) with pass1(b1) (1 MiB loads; ypool has spare
        # slots so b1's first chunks can start during the fold); fold(b1);
        # pass2(b1).
        for t in range(S_FULL // XT0):
            pass1_b0_tile(t)
        sc0 = stats_fold(0)
        for j in range(N_CH):
            pass2_chunk(0, j, sc0)
            pass1_b1_chunk(j)
        sc1 = stats_fold(1)
        for j in range(N_CH):
            pass2_chunk(1, j, sc1)

    nc.compile()
    return nc


def host_consts(w, b):
    """Host-side tiny constant tensors fed to every core."""
    w = np.asarray(w, np.float32).reshape(16)
    b = np.asarray(b, np.float32).reshape(1)
    W2 = np.stack([np.roll(w, o) for o in range(16)], axis=0)   # [O,C]
    wbd = np.kron(W2.T.copy(), np.eye(G, dtype=np.float32))     # [128,128]
    wbd = np.ascontiguousarray(wbd, np.float32)
    ident = np.eye(128, dtype=np.float32)
    sel = np.zeros((32, 2, 128), np.float32)
    for p in range(128):
        sel[p // G, 0, p] = 1.0
        sel[16 + p // G, 1, p] = 1.0
    b128 = np.full((128, 1), float(b[0]), np.float32)
    return wbd, ident, sel, b128


_NC = None
LAST_RESULTS = None


def kernel(x, w, b):
    global _NC, LAST_RESULTS
    x = np.ascontiguousarray(np.asarray(x, np.float32))
    assert x.shape == (B, C, H, W)
    if _NC is None:
        _NC = build_nc()
    wbd, ident, sel, b128 = host_consts(w, b)

    xg = x.reshape(N_CORES, BP, C, G, S_FULL)
    in_maps = [
        {
            "x": np.ascontiguousarray(xg[ci]),
            "wbd": wbd,
            "ident": ident,
            "sel": sel,
            "b128": b128,
        }
        for ci in range(N_CORES)
    ]
    res = run_bass_kernel_spmd(_NC, in_maps, core_ids=list(range(N_CORES)))
    LAST_RESULTS = res
    out = np.concatenate([r["y"].reshape(BP, C, H, W) for r in res.results], axis=0)
    return out
